# revision 3
# baseline (speedup 1.0000x reference)
"""Trainium2 Bass kernel for DeTrAttention (dense transformer MHA block).

Full op: out = softmax((q@Wq+bq)(k@Wk+bk)^T / sqrt(64)) (v@Wv+bv) @ Wo + bo
Shapes: q,k,v [B=2, S=2048, H=1024], NH=16 heads, HD=64.

Sharding (8 cores): tensor-parallel over heads within each batch group
(Titans-style).  Cores 4b..4b+3 handle batch b; core rr in the group owns
heads 4rr..4rr+3 (256 of the 1024 projection features) END-to-END: Q/K/V
projections column-sliced (no redundant compute, no collectives), attention
for its 4 heads over the full sequence, and a row-slice of the dense output
projection producing a PARTIAL output.  The host sums the 4 partials per
batch during unshard (out = sum_c partial_c + bv@Wo + bo - both biases fold
out of the device program; bq/bk are fused into the DVE psum->sbuf copies as
per-partition scalars).

On-chip layout (no on-chip transposes):
  - host passes q^T, k^T, v^T ([H, S] feature-major) and column/row slices
    of the weights; Wq/bq pre-scaled by 1/sqrt(64)
  - qp^T, kp^T [256, S] computed W-stationary; bias via DVE tensor_scalar
  - vp token-major [S, 4, 65] bf16 with an appended ones-column that makes
    the softmax denominator Z drop out of the ctx matmul (row 64)
  - scores^T[kt, qt] = kp_head.T @ qp_head (K=64 contraction); exp on the
    scalar engine straight from PSUM (|scores| <= ~8, no max subtraction)
  - ctx^T[d, qt] accumulated over kt blocks, lhsT = vp
  - normalize via DVE reciprocal + gpsimd partition-broadcast + DVE mul
  - out^T partial = Wo_slice stationary over ctx^T, interleaved per q-chunk
Projections and scores run in float32r (fp32 data at full PE rate for free
dims >= 256); attention probabilities and V are bf16.  Weights/biases are
loop-invariant and live in SBUF across timing-loop iterations.
"""

import contextlib

import numpy as np

import concourse.bass as bass
import concourse.tile as tile
from concourse import bacc, mybir
from concourse.bass_utils import run_bass_kernel_spmd

F32 = mybir.dt.float32
F32R = mybir.dt.float32r
BF16 = mybir.dt.bfloat16

B, S, H, NH = 2, 2048, 1024, 16
HD = H // NH  # 64
N_CORES = 8
CPG = N_CORES // B     # cores per batch group (4)
HPC = NH // CPG        # heads per core (4)
HF = HPC * HD          # projection features per core (256)
SQ = S                 # every core sees all query rows (for test.py compat)


def build_nc(reps=0, upto=3, ch=2, wn=256, sreps=1):
    """Build the per-core Bass program (same program on all 8 cores).

    reps > 0 wraps the body in a hardware For_i loop (timing vehicle).
    sreps > 1 statically unrolls the body instead.
    """
    s, h, hd, hpc, hf = S, H, HD, HPC, HF
    KB = h // 128          # contraction blocks over h_in (8)
    MBC = hf // 128        # feature 128-blocks per core (2)
    WN = wn                # token chunk for streaming projections
    NCH = s // WN          # chunks per input tensor
    TPW = WN // 128        # token 128-blocks per chunk
    KTB = s // 128         # key-token 128-blocks (16)
    QW = 512               # query chunk (psum free-dim)
    QC = s // QW           # query chunks (4)
    OB = h // 128          # output feature blocks (8)
    CH = ch                # ktb per attention chunk

    nc = bacc.Bacc("TRN2", target_bir_lowering=False, debug=False)

    qT = nc.dram_tensor("qT", [h, s], F32R, kind="ExternalInput").ap()
    kT = nc.dram_tensor("kT", [h, s], F32R, kind="ExternalInput").ap()
    vT = nc.dram_tensor("vT", [h, s], F32R, kind="ExternalInput").ap()
    Wq = nc.dram_tensor("Wq", [h, hf], F32R, kind="ExternalInput").ap()
    Wk = nc.dram_tensor("Wk", [h, hf], F32R, kind="ExternalInput").ap()
    Wv = nc.dram_tensor("Wv", [h, hf], F32R, kind="ExternalInput").ap()
    Wo = nc.dram_tensor("Wo", [hf, h], F32R, kind="ExternalInput").ap()
    bqk = nc.dram_tensor("bqk", [128, 2 * MBC], F32, kind="ExternalInput").ap()
    outT = nc.dram_tensor("outT", [h, s], F32, kind="ExternalOutput").ap()

    # [p, kb, cols] views (partition-major) so whole tensors load in one DMA
    qT_p = qT.rearrange("(kb p) t -> p kb t", p=128)
    kT_p = kT.rearrange("(kb p) t -> p kb t", p=128)
    vT_p = vT.rearrange("(kb p) t -> p kb t", p=128)
    Wq_p = Wq.rearrange("(kb p) o -> p kb o", p=128)
    Wk_p = Wk.rearrange("(kb p) o -> p kb o", p=128)
    Wv_p = Wv.rearrange("(kb p) o -> p kb o", p=128)
    Wo_p = Wo.rearrange("(kb p) o -> p kb o", p=128)
    outT_p = outT.rearrange("(ob p) t -> p ob t", p=128)

    with tile.TileContext(nc) as tc:
        with tc.tile_pool(name="persist", bufs=1) as persist, \
             tc.tile_pool(name="consts", bufs=1) as consts, \
             tc.tile_pool(name="stream", bufs=2) as stream, \
             tc.tile_pool(name="wqo", bufs=2) as wqo, \
             tc.tile_pool(name="exps", bufs=8) as exps, \
             tc.tile_pool(name="zrp", bufs=2) as zrp, \
             tc.tile_pool(name="ps512", bufs=2, space="PSUM") as ps512, \
             tc.tile_pool(name="ps1024", bufs=2, space="PSUM") as ps1024, \
             tc.tile_pool(name="psc", bufs=2, space="PSUM") as pscp:

            # ---- loop-invariant: weights, biases, constants (1 DMA each) ----
            ones_rep = consts.tile([128, hpc], F32)
            nc.vector.memset(ones_rep, 1.0)
            bqk_sb = consts.tile([128, 2 * MBC], F32, tag="bqk")
            nc.sync.dma_start(out=bqk_sb, in_=bqk)
            Wq_sb = consts.tile([128, KB, hf], F32R, tag="wq")
            Wk_sb = consts.tile([128, KB, hf], F32R, tag="wk")
            Wv_sb = consts.tile([128, KB, hf], F32R, tag="wv")
            Wo_sb = consts.tile([128, MBC, h], F32R, tag="wo")
            nc.sync.dma_start(out=Wk_sb, in_=Wk_p)
            nc.sync.dma_start(out=Wq_sb, in_=Wq_p)
            nc.sync.dma_start(out=Wv_sb, in_=Wv_p)
            nc.sync.dma_start(out=Wo_sb, in_=Wo_p)

            # persistent activations
            kpT = persist.tile([128, MBC, s], F32R, tag="kpT")
            qpT = persist.tile([128, MBC, s], F32R, tag="qpT")
            vp = persist.tile([128, KTB, hpc, hd + 1], BF16, tag="vp")
            ctxnT = persist.tile([128, MBC, s], F32R, tag="ctxnT")

            # ones column of vp (softmax denominator trick) is loop-invariant
            for t in range(KTB):
                nc.vector.tensor_copy(vp[:, t, :, hd:hd + 1], ones_rep)

            loop_cm = tc.For_i(0, reps, 1) if reps else contextlib.nullcontext()
            with loop_cm:
              for _srep in range(sreps):
                # ---- kp^T / qp^T projections: [hf(128 x MBC), t] ----
                for W_sb, boff, dst, src_p, tg in (
                        (Wk_sb, MBC, kpT, kT_p, "kt"),
                        (Wq_sb, 0, qpT, qT_p, "qt")):
                    for n in range(NCH):
                        t_t = stream.tile([128, KB, WN], F32R, tag=tg,
                                          name=f"{tg}_t")
                        nc.sync.dma_start(
                            out=t_t, in_=src_p[:, :, n * WN:(n + 1) * WN])
                        for mb in range(MBC):
                            ps = ps512.tile([128, WN], F32, tag="ps512")
                            for kb in range(KB):
                                nc.tensor.matmul(
                                    ps, W_sb[:, kb, mb * 128:(mb + 1) * 128],
                                    t_t[:, kb, :], start=(kb == 0),
                                    stop=(kb == KB - 1))
                            nc.vector.tensor_scalar(
                                dst[:, mb, n * WN:(n + 1) * WN], ps,
                                bqk_sb[:, boff + mb:boff + mb + 1], None,
                                mybir.AluOpType.add)

                # ---- vp projection (token-major) ----
                for n in range(NCH):
                    vt_t = stream.tile([128, KB, WN], F32R, tag="vt",
                                       name="vt_t")
                    nc.sync.dma_start(
                        out=vt_t, in_=vT_p[:, :, n * WN:(n + 1) * WN])
                    for st in range(TPW):
                        t = n * TPW + st
                        ps = ps512.tile([128, hf], F32, tag="ps512")
                        for kb in range(KB):
                            nc.tensor.matmul(
                                ps, vt_t[:, kb, st * 128:(st + 1) * 128],
                                Wv_sb[:, kb, :], start=(kb == 0),
                                stop=(kb == KB - 1))
                        nc.vector.tensor_copy(
                            vp[:, t, :, 0:hd],
                            ps.rearrange("p (hh d) -> p hh d", d=hd))

                if upto < 2:
                    # consume proj outputs so DCE keeps them (timing mode)
                    nc.sync.dma_start(out=outT_p[:, 0, :],
                                      in_=kpT[:, 0, :].bitcast(F32))
                    nc.sync.dma_start(out=outT_p[:, 1, :],
                                      in_=qpT[:, 0, :].bitcast(F32))
                    nc.sync.dma_start(out=outT_p[:, 2, 0:65],
                                      in_=vp[:, 0, 0, :])
                    continue

                # ---- attention (2 concurrent head-pair chains) + out proj,
                # per 512-query chunk.  scores -> exp -> ctx software-
                # pipelined one chunk behind so ACT streams without PE
                # ping-pong; the sibling chain fills cross-engine bubbles. ----
                for qc in range(QC):
                    qsl = slice(qc * QW, (qc + 1) * QW)
                    pscs = []
                    for m in range(MBC):
                        pool = pscp if m == 0 else ps512
                        tag = "psc" if m == 0 else "ps512"
                        pscs.append([
                            pool.tile([hd + 1, QW], F32, tag=tag,
                                      name=f"psc{m}_{j}")
                            for j in range(2)])
                    prevs = [None] * MBC
                    for cc in range(KTB // CH):
                        for m in range(MBC):
                            p1s = [ps1024.tile([128, CH, QW], F32,
                                               tag="ps1024", bufs=2,
                                               name=f"p1_{m}_{j}")
                                   for j in range(2)]
                            for i in range(CH):
                                ktb = cc * CH + i
                                for j, roff in enumerate((0, 64)):
                                    nc.tensor.matmul(
                                        p1s[j][:, i, :],
                                        kpT[roff:roff + 64, m,
                                            ktb * 128:(ktb + 1) * 128],
                                        qpT[roff:roff + 64, m, qsl],
                                        start=True, stop=True)
                            ets = []
                            for j in range(2):
                                et = exps.tile([128, CH, QW], BF16,
                                               tag="exp_t", bufs=8,
                                               name=f"et_{m}_{j}")
                                nc.scalar.activation(
                                    out=et, in_=p1s[j],
                                    func=mybir.ActivationFunctionType.Exp)
                                ets.append(et)
                            if prevs[m] is not None:
                                pcc, pets = prevs[m]
                                for j in range(2):
                                    for i in range(CH):
                                        ktb = pcc * CH + i
                                        nc.tensor.matmul(
                                            pscs[m][j],
                                            vp[:, ktb, 2 * m + j, :],
                                            pets[j][:, i, :],
                                            start=(ktb == 0),
                                            stop=(ktb == KTB - 1))
                            prevs[m] = (cc, ets)
                    for m in range(MBC):
                        pcc, pets = prevs[m]
                        for j in range(2):
                            for i in range(CH):
                                ktb = pcc * CH + i
                                nc.tensor.matmul(
                                    pscs[m][j], vp[:, ktb, 2 * m + j, :],
                                    pets[j][:, i, :], start=(ktb == 0),
                                    stop=(ktb == KTB - 1))
                            # normalize: ctxn = ctx * (1/Z); Z-broadcast on
                            # the otherwise-idle gpsimd engine
                            roff = 64 * j
                            zr = zrp.tile([1, QW], F32, tag="zr", bufs=2)
                            with nc.allow_low_precision(
                                    reason="1/Z of softmax; DVE mul"):
                                nc.vector.reciprocal(
                                    zr, pscs[m][j][hd:hd + 1, :])
                            zb = zrp.tile([hd, QW], F32, tag="zb", bufs=2)
                            nc.gpsimd.partition_broadcast(zb, zr)
                            nc.vector.tensor_mul(
                                ctxnT[roff:roff + 64, m, qsl],
                                pscs[m][j][0:hd, :], zb)

                    if upto < 3:
                        if qc == 0:
                            nc.sync.dma_start(
                                out=outT_p[:, 3, :],
                                in_=ctxnT[:, 0, :].bitcast(F32))
                        continue
                    # ---- output projection (partial: this core's 256-row
                    # slice of Wo), interleaved per query chunk ----
                    for ob in range(OB):
                        po = ps512.tile([128, QW], F32, tag="ps512")
                        for mb in range(MBC):
                            nc.tensor.matmul(
                                po, Wo_sb[:, mb, ob * 128:(ob + 1) * 128],
                                ctxnT[:, mb, qsl], start=(mb == 0),
                                stop=(mb == MBC - 1))
                        ot = wqo.tile([128, QW], F32, tag="ot")
                        nc.vector.tensor_copy(ot, po)
                        nc.sync.dma_start(out=outT_p[:, ob, qsl], in_=ot)

    nc.compile()
    return nc


def shard_inputs(q, k, v, Wq, bq, Wk, bk, Wv, bv, Wo, bo):
    """Host-side sharding: per-core input dicts (numpy, fp32, contiguous)."""
    scale = np.float32(1.0 / np.sqrt(HD))
    c32 = lambda a: np.ascontiguousarray(a, dtype=np.float32)
    q, k, v = np.asarray(q), np.asarray(k), np.asarray(v)
    Wq_s = c32(Wq) * scale
    bq_s = c32(bq) * scale
    Wk_c, bk_c, Wv_c, Wo_c = c32(Wk), c32(bk), c32(Wv), c32(Wo)
    qT_b = [c32(q[b].T) for b in range(B)]
    kT_b = [c32(k[b].T) for b in range(B)]
    vT_b = [c32(v[b].T) for b in range(B)]
    in_maps = []
    for c in range(N_CORES):
        b, rr = c // CPG, c % CPG
        hs = slice(rr * HF, (rr + 1) * HF)
        # per-partition bias columns: col mb of bqk = bias[hs][mb*128:...]
        bqk_cols = np.concatenate([
            bq_s[hs].reshape(HF // 128, 128).T,
            bk_c[hs].reshape(HF // 128, 128).T], axis=1)
        in_maps.append({
            "qT": qT_b[b], "kT": kT_b[b], "vT": vT_b[b],
            "Wq": c32(Wq_s[:, hs]), "Wk": c32(Wk_c[:, hs]),
            "Wv": c32(Wv_c[:, hs]), "Wo": c32(Wo_c[hs, :]),
            "bqk": c32(bqk_cols),
        })
    return in_maps


_NC_CACHE = {}


def get_nc():
    if "nc" not in _NC_CACHE:
        _NC_CACHE["nc"] = build_nc()
    return _NC_CACHE["nc"]


def kernel(q, k, v, Wq, bq, Wk, bk, Wv, bv, Wo, bo):
    q, k, v = np.asarray(q), np.asarray(k), np.asarray(v)
    in_maps = shard_inputs(q, k, v, Wq, bq, Wk, bk, Wv, bv, Wo, bo)
    nc = get_nc()
    res = run_bass_kernel_spmd(nc, in_maps, core_ids=list(range(N_CORES)))
    # host-side unshard: sum the 4 per-head-group partials per batch and
    # fold in the biases that commute with the output projection
    # ((ctx + bv) @ Wo + bo = ctx @ Wo + (bv @ Wo + bo)).
    bias = (np.asarray(bv, np.float32) @ np.asarray(Wo, np.float32)
            + np.asarray(bo, np.float32))
    out = np.empty((B, S, H), dtype=np.float32)
    for b in range(B):
        acc = res.results[b * CPG]["outT"].T.astype(np.float32).copy()
        for rr in range(1, CPG):
            acc += res.results[b * CPG + rr]["outT"].T
        out[b] = acc + bias
    return out
